# revision 11
# baseline (speedup 1.0000x reference)
"""RNN-T joint network kernel for 8 Trainium2 NeuronCores.

out[b,t,u,:] = W2 @ tanh(W1e @ enc[b,t] + W1d @ dec[b,u] + b1) + b2

Shapes: B=4, T=200, U=100, D=512, H=1024, O=512.
Sharding: T split 8 ways (25 t's per core); dec + weights replicated.

All matmul inputs are bf16 (converted on host): halves input DMA, enables
FWL fast weight loads, and removes the fp32r cast pass.  Output is written
bf16 and upcast on host (norm rel err ~4e-3, well under the 2e-2 gate).

Per-core device program:
  Warmup: ~36 small dummy matmuls keep the PE busy through the input-DMA
          window so the HAM clock-gate is at 8/8 (2.4 GHz) for phase 1.
  Phase 1: per hk chunk (enc/dec interleaved; w1 arrives in quarter DMAs
           ordered by need): enc_hT = W1e @ encT (+b1), dec_hT = W1d @ decT.
           PSUM evacs run on ACT so the DVE FIFO holds only builds.
  Phase 2: for each chunk (b, up to 5 t's) = up to 500 rows:
           k=0..5: s[k] = dec_hT[k] (+) enc_hT bcast (DVE TT, stride-0 APs,
           fp32 in -> bf16 out) then one in-place tanh over [128, 6*rows]
           (ACT); k=6,7 are built fused on ACT via activation(Tanh,
           bias=ench col) per t, straight from dech.  Then 4x8 accumulating
           bf16 matmuls against W2 chunks (oc-major layout) -> psum
           [128, 512]; +b2 on the psum->sbuf copy (oc0,1 ACT / oc2,3 DVE)
           into one [128, 4*rows] staging tile; ONE output DMA per chunk
           (rings alternate per chunk).
"""

from contextlib import ExitStack

import numpy as np
import ml_dtypes

import concourse.bacc as bacc
import concourse.bass as bass
import concourse.mybir as mybir
import concourse.tile as tile
from concourse.bass_utils import run_bass_kernel_spmd

F32 = mybir.dt.float32
BF16 = mybir.dt.bfloat16
NPBF16 = ml_dtypes.bfloat16

B, T, U, D, H, O = 4, 200, 100, 512, 1024, 512
NCORES = 8
TLOC = T // NCORES            # 25 t's per core
PAIRS = B * TLOC              # 100 (b,t) pairs per core
TCH = 5                       # t's per inner chunk
CHROWS = TCH * U              # 500 rows per chunk
NCH = TLOC // TCH             # 5 chunks per b
ROWS = PAIRS * U              # 10000 output rows per core
DK = D // 128                 # 4 contraction chunks for phase 1
HK = H // 128                 # 8 h chunks

_CACHE = {}


def _build():
    nc = bacc.Bacc("TRN2", target_bir_lowering=False, debug=False,
                   num_devices=NCORES)
    # inputs arrive pre-interleaved in SBUF layout: [128, nchunk*width],
    # partition p holding chunk k's row (k*128+p) at cols [k*width, ...)
    encT = nc.dram_tensor("encT", [128, DK * PAIRS], BF16, kind="ExternalInput")
    decT = nc.dram_tensor("decT", [128, DK * B * U], BF16, kind="ExternalInput")
    # w1 layouts are hk-major: [128, hk, dk, 128] -> quarter DMAs (2 hk each)
    # are contiguous; phase-1 hk work depends only on its quarter
    w1eT = nc.dram_tensor("w1eT", [128, HK * D], BF16, kind="ExternalInput")
    w1dT = nc.dram_tensor("w1dT", [128, HK * D], BF16, kind="ExternalInput")
    # w2 layout is oc-major: [128, oc, hk, 128] -> the first matmul group
    # (oc=0) only needs the first half
    w2T = nc.dram_tensor("w2T", [128, HK * O], BF16, kind="ExternalInput")
    b1r = nc.dram_tensor("b1r", [128, HK], F32, kind="ExternalInput")
    b2c = nc.dram_tensor("b2c", [128, O // 128], F32, kind="ExternalInput")
    out = nc.dram_tensor("out", [O, ROWS], BF16, kind="ExternalOutput")

    BU = B * U
    QHK = 2                    # hk per w1 quarter
    with tile.TileContext(nc) as tc, ExitStack() as ctx:
        consts = ctx.enter_context(tc.tile_pool(name="consts", bufs=1))
        spool = ctx.enter_context(tc.tile_pool(name="spool", bufs=4))
        opool = ctx.enter_context(tc.tile_pool(name="opool", bufs=4))
        psB = ctx.enter_context(tc.tile_pool(name="psB", bufs=8, space="PSUM"))

        # ---- PE warmup: dummy matmuls so HAM un-throttles before phase 1 ----
        warm = consts.tile([128, 128], BF16, name="warm")
        nc.vector.memset(warm[:], 0.0)
        for _ in range(36):
            pw = psB.tile([128, 512], F32, tag="psB", name="pw")
            nc.tensor.matmul(pw[:, :128], lhsT=warm[:], rhs=warm[:],
                             start=True, stop=True)

        # ---- load inputs: both HWDGE rings, strict need-order, w2 last ----
        w1e_s = [consts.tile([128, QHK * D], BF16, name=f"w1e{i}")
                 for i in range(4)]
        w1d_s = [consts.tile([128, QHK * D], BF16, name=f"w1d{i}")
                 for i in range(4)]
        w2_s = [consts.tile([128, 2 * H], BF16, name=f"w2{i}")
                for i in range(2)]
        encT_s = consts.tile([128, DK * PAIRS], BF16)
        decT_s = consts.tile([128, DK * BU], BF16)
        b1_s = consts.tile([128, HK], F32)
        b2c_s = consts.tile([128, O // 128], F32)
        QW = QHK * D
        nc.sync.dma_start(encT_s[:], encT[:])
        nc.scalar.dma_start(decT_s[:], decT[:])
        nc.sync.dma_start(w1e_s[0][:], w1eT[:, 0 * QW:1 * QW])
        nc.scalar.dma_start(w1d_s[0][:], w1dT[:, 0 * QW:1 * QW])
        nc.sync.dma_start(w1d_s[1][:], w1dT[:, 1 * QW:2 * QW])
        nc.scalar.dma_start(w1e_s[1][:], w1eT[:, 1 * QW:2 * QW])
        nc.sync.dma_start(w1e_s[2][:], w1eT[:, 2 * QW:3 * QW])
        nc.scalar.dma_start(w1d_s[2][:], w1dT[:, 2 * QW:3 * QW])
        nc.sync.dma_start(w1d_s[3][:], w1dT[:, 3 * QW:4 * QW])
        nc.scalar.dma_start(w1e_s[3][:], w1eT[:, 3 * QW:4 * QW])
        nc.sync.dma_start(b1_s[:], b1r[:])
        nc.sync.dma_start(b2c_s[:], b2c[:])
        nc.sync.dma_start(w2_s[0][:], w2T[:, :2 * H])
        nc.scalar.dma_start(w2_s[1][:], w2T[:, 2 * H:])

        def w1e_ap(hk, dk):
            return w1e_s[hk // QHK][:, (hk % QHK) * D + dk * 128:][:, :128]

        def w1d_ap(hk, dk):
            return w1d_s[hk // QHK][:, (hk % QHK) * D + dk * 128:][:, :128]

        def w2_ap(k, oc):
            return w2_s[oc // 2][:, (oc % 2) * H + k * 128:][:, :128]

        # ---- phase 1: enc_hT (+b1) and dec_hT, interleaved per hk ----
        # k-PAIR tiles (matching the w1 quarter DMAs): fine-grained enough
        # that phase-2 builds start as each pair is ready, and wide enough
        # that each build covers two k's in one DVE instruction.
        # evacs live on ACT so the DVE FIFO holds only builds.
        NP_ = HK // QHK
        ench_p = [consts.tile([128, QHK * PAIRS], F32, name=f"ench{q}")
                  for q in range(NP_)]
        dech_p = [consts.tile([128, QHK * BU], F32, name=f"dech{q}")
                  for q in range(NP_)]
        for hk in range(HK):
            q, r = hk // QHK, hk % QHK
            pe = psB.tile([128, 512], F32, tag="psB", name="pe")
            pe = pe[:, :PAIRS]
            for dk in range(DK):
                nc.tensor.matmul(
                    pe[:],
                    lhsT=w1e_ap(hk, dk),
                    rhs=encT_s[:, dk * PAIRS:(dk + 1) * PAIRS],
                    start=(dk == 0), stop=(dk == DK - 1),
                )
            nc.scalar.activation(ench_p[q][:, r * PAIRS:(r + 1) * PAIRS], pe[:],
                                 mybir.ActivationFunctionType.Identity,
                                 bias=b1_s[:, hk:hk + 1])
            pd = psB.tile([128, 512], F32, tag="psB", name="pd")
            pd = pd[:, :BU]
            for dk in range(DK):
                nc.tensor.matmul(
                    pd[:],
                    lhsT=w1d_ap(hk, dk),
                    rhs=decT_s[:, dk * BU:(dk + 1) * BU],
                    start=(dk == 0), stop=(dk == DK - 1),
                )
            nc.scalar.activation(dech_p[q][:, r * BU:(r + 1) * BU], pd[:],
                                 mybir.ActivationFunctionType.Identity)

        # ---- phase 2: chunks of (b, up to 5 t's) ----
        # small lead-in chunks shorten the build+tanh fill before the first
        # big matmul group; small drain chunks shorten the output tail
        chunks = []
        for b in range(B):
            if b == 0:
                sizes = [1, 4] + [TCH] * 4
            elif b == B - 1:
                sizes = [TCH] * 4 + [3, 2]
            else:
                sizes = [TCH] * NCH
            t0c = 0
            for tch in sizes:
                chunks.append((b, t0c, tch))
                t0c += tch
        for ci, (b, t0c, tch) in enumerate(chunks):
            rows_c = tch * U
            c0 = b * TLOC + t0c
            s_t = spool.tile([128, HK * CHROWS], BF16, tag="s")
            # DVE broadcast-add build (fp32 -> bf16), one instr per k-pair
            for q in range(HK // QHK):
                in0 = dech_p[q][:].rearrange("p (k u) -> p k u", k=QHK)
                in0 = in0[:, :, b * U:(b + 1) * U].rearrange(
                    "p k (a u) -> p k a u", a=1)
                in1 = ench_p[q][:].rearrange("p (k c) -> p k c", k=QHK)
                in1 = in1[:, :, c0:c0 + tch].rearrange(
                    "p k (t a) -> p k t a", a=1)
                bc0, bc1 = bass.broadcast_tensor_aps(in0, in1)
                outap = s_t[:, q * QHK * CHROWS:(q + 1) * QHK * CHROWS]
                outap = outap.rearrange("p (k c) -> p k c", k=QHK)
                outap = outap[:, :, :rows_c].rearrange(
                    "p k (t u) -> p k t u", t=tch)
                nc.vector.tensor_tensor(outap, bc0, bc1, mybir.AluOpType.add)
            # one in-place tanh over all k
            s_used = s_t[:].rearrange("p (k c) -> p k c", k=HK)[:, :, :rows_c]
            nc.scalar.activation(s_used, s_used,
                                 mybir.ActivationFunctionType.Tanh)
            row0 = b * (TLOC * U) + t0c * U
            # swapped matmul: W2 blocks stationary, s moving -> psum holds
            # out^T [o-chunk, rows]; b2 folds into the psum->sbuf copy as a
            # per-partition bias; all 4 oc slices land in one staging tile
            # so the chunk's output is a single DMA.
            ot = opool.tile([128, 4 * CHROWS], BF16, tag="ot")
            for oc in range(O // 128):
                ps = psB.tile([128, 512], F32, tag="psB")
                for k in range(HK):
                    nc.tensor.matmul(
                        ps[:, :rows_c],
                        lhsT=w2_ap(k, oc),
                        rhs=s_t[:, k * CHROWS: k * CHROWS + rows_c],
                        start=(k == 0), stop=(k == HK - 1),
                    )
                oslice = ot[:, oc * CHROWS: oc * CHROWS + rows_c]
                if oc < 2:
                    nc.scalar.activation(
                        oslice, ps[:, :rows_c],
                        mybir.ActivationFunctionType.Identity,
                        bias=b2c_s[:, oc:oc + 1])
                else:
                    nc.vector.tensor_scalar_add(
                        oslice, ps[:, :rows_c], b2c_s[:, oc:oc + 1])
            dst = out[:, row0:row0 + rows_c].rearrange(
                "(oc p) r -> p oc r", p=128)
            src = ot[:].rearrange("p (oc c) -> p oc c", oc=4)[:, :, :rows_c]
            ring = nc.sync if ci % 2 == 0 else nc.scalar
            ring.dma_start(dst, src)
    nc.compile()
    return nc


def kernel(enc_state, dec_state, W1, b1, W2, b2, _trace=False):
    enc_state = np.ascontiguousarray(enc_state, dtype=np.float32)
    dec_state = np.ascontiguousarray(dec_state, dtype=np.float32)
    W1 = np.asarray(W1, dtype=np.float32)
    b1 = np.asarray(b1, dtype=np.float32)
    W2 = np.asarray(W2, dtype=np.float32)
    b2 = np.asarray(b2, dtype=np.float32)

    if "nc" not in _CACHE:
        _CACHE["nc"] = _build()
    nc = _CACHE["nc"]

    def chunk128(a):
        # [n*128, w] -> [128, n*w]: partition p holds row k*128+p of chunk k
        n = a.shape[0] // 128
        return np.ascontiguousarray(
            a.reshape(n, 128, a.shape[1]).transpose(1, 0, 2).reshape(128, -1))

    def hk_major(w):
        # chunk128 of [D, H] -> [128, dk, hk, 128]; reorder to [128, hk, dk, 128]
        c = chunk128(w)  # [128, DK*H]
        return np.ascontiguousarray(
            c.reshape(128, DK, HK, 128).transpose(0, 2, 1, 3).reshape(128, -1))

    def oc_major(w2c):
        # chunk128 of [H, O] = [128, hk, oc, 128]; reorder to [128, oc, hk, 128]
        return np.ascontiguousarray(
            w2c.reshape(128, HK, O // 128, 128).transpose(0, 2, 1, 3)
            .reshape(128, -1))

    decT = chunk128(dec_state.reshape(B * U, D).T).astype(NPBF16)
    w1eT = hk_major(W1[:, :D].T.astype(NPBF16))
    w1dT = hk_major(W1[:, D:].T.astype(NPBF16))
    w2T = oc_major(chunk128(W2.T.astype(NPBF16)))
    b1r = np.ascontiguousarray(b1.reshape(HK, 128).T)                   # [128, HK]
    b2cm = np.ascontiguousarray(b2.reshape(O // 128, 128).T)            # [128, 4]

    in_maps = []
    for c in range(NCORES):
        enc_c = enc_state[:, c * TLOC:(c + 1) * TLOC, :].reshape(PAIRS, D)
        encT_c = chunk128(enc_c.T).astype(NPBF16)                       # [128, 4*100]
        in_maps.append({
            "encT": encT_c, "decT": decT, "w1eT": w1eT, "w1dT": w1dT,
            "w2T": w2T, "b1r": b1r, "b2c": b2cm,
        })

    res = run_bass_kernel_spmd(nc, in_maps, list(range(NCORES)), trace=_trace)
    out = np.empty((B, T, U, O), dtype=np.float32)
    for c in range(NCORES):
        # device output is transposed: [O, ROWS]
        out[:, c * TLOC:(c + 1) * TLOC] = (
            res.results[c]["out"].astype(np.float32).T.reshape(B, TLOC, U, O))
    if _trace:
        kernel.last_results = res
    return out
